# revision 3
# baseline (speedup 1.0000x reference)
"""Depth-to-space (CRD order) kernel for Trainium2, 8 NeuronCores.

in:  (32, 9, 512, 512) f32, channel c = r*3+s encodes (row_off, col_off)
out: (32, 1, 1536, 1536) f32 with out[b,0,3i+r,3j+s] = in[b,3r+s,i,j]

Sharding: data-parallel over batch, 4 batches per core, no communication.

Design notes (HW-measured on trn2):
- The 16 SDMA engines are the bottleneck; they stream descriptors serially
  and their per-engine payload rate depends on the per-partition contiguous
  run length: HBM reads ~23.2 / 25.9 / 26.8 GB/s at 2K/8K/32K runs, HBM
  writes ~26.2-26.8 at >=6K runs.  Engine busy time tracks the LARGER side
  of each descriptor, so bf16-in-DMA cast (SWDGE) does not help: the f32
  HBM side still bounds it (verified: cast stores' busy time == f32 ones).
- Per-core layout: partition p holds image rows 4p..4p+3, so loads pull a
  channel-triple with 8KB contiguous runs and stores write output
  row-triples (3*(4p+d)+{0,1,2}) as 18KB contiguous runs.
- Loads ride the sync HWDGE ring, stores the scalar ring (separate FIFOs so
  a waiting store never blocks ready loads); DVE interleaves in between.
- ~9us fixed preamble (all-engine sem rendezvous etc) + ~179us DMA floor.
  Measured 192.4us/core on a quiet device (baseline fine-grained kernel:
  197.9us).  Keep DMA-issuing engines free of compute ops (ACT copies on
  the scalar ring head-of-line-block store dispatch), and keep the
  per-(d) copy->store chains in issue order (clustering stores at the
  batch end stalls the pipeline).
"""

import sys

import numpy as np

_B, _C, _H, _W = 32, 9, 512, 512
_K = 3
_NCORES = 8
_BLOC = _B // _NCORES  # 4

_PROG = None


def _ensure_path():
    try:
        import concourse.bass  # noqa: F401
    except ImportError:
        sys.path.insert(0, "/opt/trn_rl_repo")


def _build():
    import concourse.bacc as bacc
    import concourse.mybir as mybir
    from concourse import tile

    f32 = mybir.dt.float32
    nc = bacc.Bacc(None)
    x = nc.declare_dram_parameter("x", [_BLOC, _C, _H, _W], f32, isOutput=False)
    y = nc.declare_dram_parameter("y", [_BLOC, _K * _H, _K * _W], f32, isOutput=True)

    P = 128
    RP = 4  # image rows per partition
    KW = _K * _W  # 1536

    with tile.TileContext(nc) as tc:
        with (
            tc.tile_pool(name="tin", bufs=5) as pin,
            tc.tile_pool(name="tout", bufs=4) as pout,
        ):
            for b in range(_BLOC):
                tins = []
                for g in range(_K):
                    tin = pin.tile([P, _K * RP * _W], f32, name="tin")
                    nc.sync.dma_start(
                        out=tin[:].rearrange("p (c dj) -> p c dj", c=_K),
                        in_=x[b, _K * g : _K * (g + 1), :, :].rearrange(
                            "c (p d) j -> p c (d j)", d=RP
                        ),
                    )
                    tins.append(tin[:].rearrange("p (c d j) -> p c d j", c=_K, d=RP))
                ydst = y[b, :, :].rearrange("(p q r) w -> q p (r w)", q=RP, r=_K)

                def _copy(tout, d, r):
                    # out[p, 3j+s] = x[b, 3r+s, 4p+d, j]
                    nc.vector.tensor_copy(
                        out=tout[:, r * KW : (r + 1) * KW].rearrange(
                            "p (j s) -> p j s", s=_K
                        ),
                        in_=tins[r][:, :, d, :].rearrange("p s j -> p j s"),
                    )

                if b < _BLOC - 1:
                    for d in range(RP):
                        tout = pout.tile([P, _K * KW], f32, name="tout")
                        for r in range(_K):
                            _copy(tout, d, r)
                        nc.scalar.dma_start(out=ydst[d], in_=tout[:])
                else:
                    # last batch: half-split stores so the store ring stays fed
                    # across the loads->stores transition.  The first halves
                    # depend only on the g0/g1 copies (ready before the final
                    # load lands); after it, each remaining half is gated by a
                    # single 0.9us copy instead of a 2.7us chain.
                    HS = _K * KW // 2  # 2304
                    touts = [pout.tile([P, _K * KW], f32, name="tout") for _ in range(RP)]
                    for d in range(RP):
                        _copy(touts[d], d, 0)
                        _copy(touts[d], d, 1)
                    for d in range(RP):
                        nc.scalar.dma_start(out=ydst[d][:, :HS], in_=touts[d][:, :HS])
                    for d in range(RP):
                        _copy(touts[d], d, 2)
                        nc.scalar.dma_start(out=ydst[d][:, HS:], in_=touts[d][:, HS:])
    return nc


def _run(x_full, trace=False, **spmd_kwargs):
    """x_full: (32, 9, 512, 512) f32 ndarray. Returns (out, BassKernelResults)."""
    global _PROG
    _ensure_path()
    from concourse.bass_utils import run_bass_kernel_spmd

    if _PROG is None:
        _PROG = _build()
        if not _PROG.is_finalized():
            _PROG.finalize()
    in_maps = [
        {"x": np.ascontiguousarray(x_full[i * _BLOC : (i + 1) * _BLOC])}
        for i in range(_NCORES)
    ]
    res = run_bass_kernel_spmd(
        _PROG, in_maps, core_ids=list(range(_NCORES)), trace=trace, **spmd_kwargs
    )
    out = np.concatenate([np.asarray(r["y"]) for r in res.results], axis=0)
    return out.reshape(_B, 1, _K * _H, _K * _W), res


def kernel(**inputs):
    x = np.ascontiguousarray(np.asarray(inputs["inputs"], dtype=np.float32))
    k = int(np.asarray(inputs.get("kernel_size", _K)))
    assert k == _K, f"kernel hardcodes kernel_size=3, got {k}"
    assert x.shape == (_B, _C, _H, _W), x.shape
    out, _ = _run(x)
    return out
